# revision 5
# baseline (speedup 1.0000x reference)
"""BellmanFord GNN message-passing layer on 8 Trainium2 NeuronCores.

Reference computation (all f32):
    x   = h[src] + edge_rel_emb          # [E, D] gather
    hid = relu(x @ W1 + b1)              # [E, D]
    msg = hid @ W2 + b2                  # [E, D]
    agg = segment_sum(msg, dst, N)       # [N, D]
    out = h + agg

Strategy:
  - Shard edges across the 8 cores by *destination node range* (N/8 = 1250
    nodes per core) so each core owns its output slice outright -- no
    cross-core reduction needed.
  - Within a core, edges are sorted by dst and grouped into 128-node
    "windows"; each window's edges are padded to a multiple of 256 so all 8
    cores share one SPMD program (per-window tile counts = max over cores).
  - Per 256-edge macrotile: indirect-DMA gather (x = rel; x += h[src]),
    PE-transpose x -> xT, hidT = relu(W1^T xT + b1) and msgT = W2^T hidT + b2
    in the transposed domain (biases become per-partition ACT biases),
    PE-transpose msgT -> msg, then scatter-sum via a selection-matrix matmul
    (S[e, n] = [dst_local[e] == n]) accumulating into a per-window PSUM bank.
  - On window close: out = h + agg via DVE, DMA to the output slice.
  - Matmuls/transposes run in float32r (~1e-4 relative error, 4x faster than
    fp32 on the PE array).
"""

import sys

sys.path.insert(0, "/opt/trn_rl_repo")

import numpy as np

import concourse.bass as bass
import concourse.mybir as mybir
import concourse.tile as tile
from concourse import bacc
from concourse.bass_utils import run_bass_kernel_spmd
from concourse.masks import make_identity

P = 128
D = 256
N_CORES = 8
ET = 256  # edges per macrotile (2 x P)
WIN = P  # nodes per scatter window
F32 = mybir.dt.float32
F32R = mybir.dt.float32r
I32 = mybir.dt.int32
AF = mybir.ActivationFunctionType

_CACHE = {}
TRACE = False
TRACE_DIR = "/tmp/ktrace"


def _build_program(n_nodes, tiles_per_window, has_b1, has_b2):
    """Build the SPMD Bass program. Identical for all 8 cores.

    tiles_per_window: list of macrotile counts, one per 128-node window of the
    per-core node slice (node slice size = n_nodes // 8, padded to 128).
    """
    npc = n_nodes // N_CORES  # nodes per core
    n_win = len(tiles_per_window)
    n_tiles = int(sum(tiles_per_window))
    npc_pad = n_win * WIN

    nc = bacc.Bacc("TRN2", target_bir_lowering=False, debug=False,
                   num_devices=N_CORES)

    h_d = nc.dram_tensor("h", [n_nodes, D], F32, kind="ExternalInput").ap()
    hs_d = nc.dram_tensor("h_slice", [npc_pad, D], F32,
                          kind="ExternalInput").ap()
    rel_d = nc.dram_tensor("rel", [n_tiles * ET, D], F32,
                           kind="ExternalInput").ap()
    src_d = nc.dram_tensor("srcT", [P, 2 * n_tiles], I32,
                           kind="ExternalInput").ap()
    dst_d = nc.dram_tensor("dstT", [P, 2 * n_tiles], F32,
                           kind="ExternalInput").ap()
    w1_d = nc.dram_tensor("w1", [D, D], F32, kind="ExternalInput").ap()
    w2_d = nc.dram_tensor("w2", [D, D], F32, kind="ExternalInput").ap()
    b1_d = nc.dram_tensor("b1", [D], F32, kind="ExternalInput").ap()
    b2_d = nc.dram_tensor("b2", [D], F32, kind="ExternalInput").ap()
    out_d = nc.dram_tensor("out", [npc_pad, D], F32, kind="ExternalOutput").ap()

    with tile.TileContext(nc) as tc:
        with (
            tc.tile_pool(name="consts", bufs=1) as cb,
            tc.tile_pool(name="x", bufs=3) as x_pool,
            tc.tile_pool(name="xT", bufs=3) as xT_pool,
            tc.tile_pool(name="hidT", bufs=3) as hidT_pool,
            tc.tile_pool(name="msgT", bufs=3) as msgT_pool,
            tc.tile_pool(name="msg", bufs=3) as msg_pool,
            tc.tile_pool(name="S", bufs=3) as s_pool,
            tc.tile_pool(name="hw", bufs=2) as h_pool,
            tc.tile_pool(name="outw", bufs=2) as out_pool,
            tc.tile_pool(name="pst", bufs=2, space="PSUM") as ps_t,  # transposes
            tc.tile_pool(name="ps1", bufs=2, space="PSUM") as ps_1,  # m1 out
            tc.tile_pool(name="ps2", bufs=2, space="PSUM") as ps_2,  # m2 out
            tc.tile_pool(name="psA", bufs=2, space="PSUM") as ps_a,  # agg
        ):
            # ---- constants ----
            ident_f = cb.tile([P, P], F32)
            make_identity(nc, ident_f[:])
            ident_r = cb.tile([P, P], F32R)
            nc.vector.tensor_copy(ident_r[:], ident_f[:])

            iota_i = cb.tile([P, P], I32)
            nc.gpsimd.iota(iota_i[:], pattern=[[1, P]], base=0,
                           channel_multiplier=0)
            iota_f = cb.tile([P, P], F32)
            nc.vector.tensor_copy(iota_f[:], iota_i[:])

            # weights as lhsT layout [ki, ko, m] in f32r
            w1_f = cb.tile([P, 2, D], F32)
            nc.sync.dma_start(w1_f[:], w1_d.rearrange("(ko ki) m -> ki ko m",
                                                      ki=P))
            w1_r = cb.tile([P, 2, D], F32R)
            nc.vector.tensor_copy(w1_r[:], w1_f[:])
            w2_f = cb.tile([P, 2, D], F32)
            nc.sync.dma_start(w2_f[:], w2_d.rearrange("(ko ki) m -> ki ko m",
                                                      ki=P))
            w2_r = cb.tile([P, 2, D], F32R)
            nc.vector.tensor_copy(w2_r[:], w2_f[:])

            b1_sb = cb.tile([P, 2], F32)
            nc.sync.dma_start(b1_sb[:], b1_d.rearrange("(m p) -> p m", p=P))
            b2_sb = cb.tile([P, 2], F32)
            nc.sync.dma_start(b2_sb[:], b2_d.rearrange("(m p) -> p m", p=P))

            # all src / dstloc indices in one DMA each
            src_sb = cb.tile([P, 2 * n_tiles], I32)
            nc.sync.dma_start(src_sb[:], src_d)
            dst_sb = cb.tile([P, 2 * n_tiles], F32)
            nc.sync.dma_start(dst_sb[:], dst_d)

            t = 0  # global macrotile index
            for w in range(n_win):
                tw = tiles_per_window[w]
                # nodes [w*WIN, (w+1)*WIN) of this core's slice
                agg_ps = ps_a.tile([P, D], F32, name="agg_ps") if tw > 0 else None
                n_mm = 2 * tw  # scatter matmuls in this window's group
                mm_i = 0
                for _ in range(tw):
                    # ---- x = rel ; x += h[src] (gather-add) ----
                    x_sb = x_pool.tile([P, 2, D], F32)
                    nc.sync.dma_start(
                        x_sb[:],
                        rel_d[t * ET:(t + 1) * ET].rearrange(
                            "(j p) d -> p j d", p=P),
                    )
                    for j in range(2):
                        nc.gpsimd.indirect_dma_start(
                            out=x_sb[:, j],
                            out_offset=None,
                            in_=h_d,
                            in_offset=bass.IndirectOffsetOnAxis(
                                ap=src_sb[:, 2 * t + j:2 * t + j + 1], axis=0),
                            compute_op=mybir.AluOpType.add,
                        )

                    # ---- transpose x -> xT [ki, k, e] ----
                    xT_ps = ps_t.tile([P, 2, ET], F32, tag="trans")
                    for j in range(2):
                        for k in range(2):
                            nc.tensor.transpose(
                                xT_ps[:, k, j * P:(j + 1) * P],
                                x_sb[:, j, k * P:(k + 1) * P],
                                ident_f[:],
                            )
                    xT_sb = xT_pool.tile([P, 2, ET], F32R)
                    nc.vector.tensor_copy(xT_sb[:], xT_ps[:])

                    # ---- m1: hidT[m] = sum_k W1[k,m]^T @ xT[k] ----
                    hidT_ps = ps_1.tile([P, 2, ET], F32)
                    for m in range(2):
                        for k in range(2):
                            nc.tensor.matmul(
                                hidT_ps[:, m],
                                lhsT=w1_r[:, k, m * P:(m + 1) * P],
                                rhs=xT_sb[:, k],
                                start=(k == 0),
                                stop=(k == 1),
                            )
                    # relu(+b1) with f32r cast, per-partition bias
                    hidT_sb = hidT_pool.tile([P, 2, ET], F32R)
                    for m in range(2):
                        nc.scalar.activation(
                            hidT_sb[:, m], hidT_ps[:, m], AF.Relu,
                            bias=b1_sb[:, m:m + 1] if has_b1 else 0.0,
                        )

                    # ---- m2: msgT[m] = sum_k W2[k,m]^T @ hidT[k] ----
                    msgT_ps = ps_2.tile([P, 2, ET], F32)
                    for m in range(2):
                        for k in range(2):
                            nc.tensor.matmul(
                                msgT_ps[:, m],
                                lhsT=w2_r[:, k, m * P:(m + 1) * P],
                                rhs=hidT_sb[:, k],
                                start=(k == 0),
                                stop=(k == 1),
                            )
                    msgT_sb = msgT_pool.tile([P, 2, ET], F32R)
                    if has_b2:
                        for m in range(2):
                            nc.vector.tensor_scalar(
                                out=msgT_sb[:, m], in0=msgT_ps[:, m],
                                scalar1=b2_sb[:, m:m + 1], scalar2=None,
                                op0=mybir.AluOpType.add,
                            )
                    else:
                        nc.vector.tensor_copy(msgT_sb[:], msgT_ps[:])

                    # ---- transpose msgT -> msg [p=e(j), d] ----
                    msg_ps = ps_t.tile([P, 2, ET], F32R, tag="trans")
                    for j in range(2):
                        for k in range(2):
                            nc.tensor.transpose(
                                msg_ps[:, j, k * P:(k + 1) * P],
                                msgT_sb[:, k, j * P:(j + 1) * P],
                                ident_r[:],
                            )
                    msg_sb = msg_pool.tile([P, 2, ET], F32R)
                    nc.vector.tensor_copy(msg_sb[:], msg_ps[:])

                    # ---- selection matrices + scatter matmul ----
                    s_sb = s_pool.tile([P, 2, P], F32R)
                    for j in range(2):
                        nc.gpsimd.tensor_scalar(
                            out=s_sb[:, j], in0=iota_f[:],
                            scalar1=dst_sb[:, 2 * t + j:2 * t + j + 1],
                            scalar2=None,
                            op0=mybir.AluOpType.is_equal,
                        )
                    for j in range(2):
                        nc.tensor.matmul(
                            agg_ps[:],
                            lhsT=s_sb[:, j],
                            rhs=msg_sb[:, j],
                            start=(mm_i == 0),
                            stop=(mm_i == n_mm - 1),
                            skip_group_check=True,
                        )
                        mm_i += 1
                    t += 1

                # ---- window close: out = h + agg ----
                h_sb = h_pool.tile([P, D], F32)
                nc.sync.dma_start(h_sb[:], hs_d[w * WIN:(w + 1) * WIN])
                out_sb = out_pool.tile([P, D], F32)
                if tw > 0:
                    nc.vector.tensor_add(out_sb[:], agg_ps[:], h_sb[:])
                else:
                    nc.vector.tensor_copy(out_sb[:], h_sb[:])
                nc.sync.dma_start(out_d[w * WIN:(w + 1) * WIN], out_sb[:])

    nc.compile()
    return nc


def _prepare_shards(h, src, dst, rel, n_nodes):
    """Shard + sort + pad edges by destination range. Returns per-core input
    arrays and the shared tiles_per_window schedule."""
    npc = n_nodes // N_CORES
    n_win = (npc + WIN - 1) // WIN
    npc_pad = n_win * WIN

    cores = []
    counts = np.zeros((N_CORES, n_win), dtype=np.int64)
    for c in range(N_CORES):
        lo, hi = c * npc, (c + 1) * npc
        mask = (dst >= lo) & (dst < hi)
        idx = np.nonzero(mask)[0]
        d_c = dst[idx] - lo
        order = np.argsort(d_c, kind="stable")
        idx = idx[order]
        d_c = d_c[order]
        w_c = d_c // WIN
        counts[c] = np.bincount(w_c, minlength=n_win)
        cores.append((idx, d_c, w_c))

    tiles_per_window = [
        max(1, int(-(-counts[:, w].max() // ET))) for w in range(n_win)
    ]
    n_tiles = int(sum(tiles_per_window))
    starts = np.concatenate([[0], np.cumsum(tiles_per_window)])

    in_maps = []
    for c in range(N_CORES):
        idx, d_c, w_c = cores[c]
        src_pad = np.zeros(n_tiles * ET, dtype=np.int32)
        dloc_pad = np.full(n_tiles * ET, -1.0, dtype=np.float32)
        rel_pad = np.zeros((n_tiles * ET, D), dtype=np.float32)
        bounds = np.searchsorted(w_c, np.arange(n_win + 1))
        for w in range(n_win):
            a, b = bounds[w], bounds[w + 1]
            k = b - a
            off = int(starts[w]) * ET
            src_pad[off:off + k] = src[idx[a:b]]
            dloc_pad[off:off + k] = (d_c[a:b] - w * WIN).astype(np.float32)
            rel_pad[off:off + k] = rel[idx[a:b]]
        h_slice = np.zeros((npc_pad, D), dtype=np.float32)
        h_slice[:npc] = h[c * npc:(c + 1) * npc]
        in_maps.append({
            "h": h,
            "h_slice": h_slice,
            "rel": rel_pad,
            "srcT": np.ascontiguousarray(src_pad.reshape(2 * n_tiles, P).T),
            "dstT": np.ascontiguousarray(dloc_pad.reshape(2 * n_tiles, P).T),
        })
    return in_maps, tiles_per_window, npc, n_win


def kernel(h, edge_index, edge_rel_emb, W1, b1, W2, b2, num_nodes):
    h = np.ascontiguousarray(h, dtype=np.float32)
    rel = np.ascontiguousarray(edge_rel_emb, dtype=np.float32)
    W1 = np.ascontiguousarray(W1, dtype=np.float32)
    W2 = np.ascontiguousarray(W2, dtype=np.float32)
    b1 = np.ascontiguousarray(b1, dtype=np.float32)
    b2 = np.ascontiguousarray(b2, dtype=np.float32)
    n_nodes = int(num_nodes)
    src = np.asarray(edge_index[0]).astype(np.int64)
    dst = np.asarray(edge_index[1]).astype(np.int64)
    assert n_nodes % N_CORES == 0
    assert h.shape == (n_nodes, D)

    in_maps, tiles_per_window, npc, n_win = _prepare_shards(
        h, src, dst, rel, n_nodes)

    has_b1 = bool(np.any(b1))
    has_b2 = bool(np.any(b2))
    key = (n_nodes, tuple(tiles_per_window), has_b1, has_b2)
    if key not in _CACHE:
        _CACHE[key] = _build_program(n_nodes, tiles_per_window, has_b1, has_b2)
    nc = _CACHE[key]

    for m in in_maps:
        m["w1"] = W1
        m["w2"] = W2
        m["b1"] = b1
        m["b2"] = b2

    trace_kwargs = {}
    if TRACE:
        trace_kwargs = dict(trace=True, tmpdir=TRACE_DIR,
                            trace_cores=list(range(N_CORES)))
    res = run_bass_kernel_spmd(nc, in_maps, core_ids=list(range(N_CORES)),
                               **trace_kwargs)
    out = np.concatenate(
        [res.results[c]["out"][:npc] for c in range(N_CORES)], axis=0)

    # stash for test harnesses
    kernel.last_results = res
    return out.astype(np.float32)


# revision 6
# speedup vs baseline: 1.3327x; 1.3327x over previous
"""BellmanFord GNN message-passing layer on 8 Trainium2 NeuronCores.

Reference computation (all f32):
    x   = h[src] + edge_rel_emb          # [E, D] gather
    hid = relu(x @ W1 + b1)              # [E, D]
    msg = hid @ W2 + b2                  # [E, D]
    agg = segment_sum(msg, dst, N)       # [N, D]
    out = h + agg

Strategy:
  - Shard edges across the 8 cores by *destination node range* (N/8 = 1250
    nodes per core) so each core owns its output slice outright -- no
    cross-core reduction needed.
  - Within a core, edges are sorted by dst and grouped into 128-node
    "windows"; each window's edges are padded to a multiple of 256 so all 8
    cores share one SPMD program (per-window tile counts = max over cores).
  - Per 256-edge macrotile: indirect-DMA gather (x = rel; x += h[src]),
    PE-transpose x -> xT, hidT = relu(W1^T xT + b1) and msgT = W2^T hidT + b2
    in the transposed domain (biases become per-partition ACT biases),
    PE-transpose msgT -> msg, then scatter-sum via a selection-matrix matmul
    (S[e, n] = [dst_local[e] == n]) accumulating into a per-window PSUM bank.
  - On window close: out = h + agg via DVE, DMA to the output slice.
  - Matmuls/transposes run in float32r (~1e-4 relative error, 4x faster than
    fp32 on the PE array).
"""

import sys

sys.path.insert(0, "/opt/trn_rl_repo")

import numpy as np

import concourse.bass as bass
import concourse.mybir as mybir
import concourse.tile as tile
from concourse import bacc
from concourse.bass_utils import run_bass_kernel_spmd
from concourse.masks import make_identity

P = 128
D = 256
N_CORES = 8
ET = 256  # edges per macrotile (2 x P)
WIN = P  # nodes per scatter window
F32 = mybir.dt.float32
F32R = mybir.dt.float32r
I32 = mybir.dt.int32
AF = mybir.ActivationFunctionType

_CACHE = {}
TRACE = False
TRACE_DIR = "/tmp/ktrace"


def _build_program(n_nodes, tiles_per_window, has_b1, has_b2):
    """Build the SPMD Bass program. Identical for all 8 cores.

    tiles_per_window: list of macrotile counts, one per 128-node window of the
    per-core node slice (node slice size = n_nodes // 8, padded to 128).
    """
    npc = n_nodes // N_CORES  # nodes per core
    n_win = len(tiles_per_window)
    n_tiles = int(sum(tiles_per_window))
    npc_pad = n_win * WIN

    nc = bacc.Bacc("TRN2", target_bir_lowering=False, debug=False,
                   num_devices=N_CORES)

    h_d = nc.dram_tensor("h", [n_nodes, D], F32, kind="ExternalInput").ap()
    hs_d = nc.dram_tensor("h_slice", [npc_pad, D], F32,
                          kind="ExternalInput").ap()
    rel_d = nc.dram_tensor("rel", [n_tiles * ET, D], F32,
                           kind="ExternalInput").ap()
    src_d = nc.dram_tensor("srcT", [P, 2 * n_tiles], I32,
                           kind="ExternalInput").ap()
    dst_d = nc.dram_tensor("dstT", [P, 2 * n_tiles], F32,
                           kind="ExternalInput").ap()
    w1_d = nc.dram_tensor("w1", [D, D], F32, kind="ExternalInput").ap()
    w2_d = nc.dram_tensor("w2", [D, D], F32, kind="ExternalInput").ap()
    b1_d = nc.dram_tensor("b1", [D], F32, kind="ExternalInput").ap()
    b2_d = nc.dram_tensor("b2", [D], F32, kind="ExternalInput").ap()
    out_d = nc.dram_tensor("out", [npc_pad, D], F32, kind="ExternalOutput").ap()

    with tile.TileContext(nc) as tc:
        with (
            tc.tile_pool(name="consts", bufs=1) as cb,
            tc.tile_pool(name="x", bufs=3) as x_pool,
            tc.tile_pool(name="xT", bufs=3) as xT_pool,
            tc.tile_pool(name="hidT", bufs=3) as hidT_pool,
            tc.tile_pool(name="msgT", bufs=3) as msgT_pool,
            tc.tile_pool(name="msg", bufs=3) as msg_pool,
            tc.tile_pool(name="S", bufs=3) as s_pool,
            tc.tile_pool(name="hw", bufs=2) as h_pool,
            tc.tile_pool(name="outw", bufs=2) as out_pool,
            tc.tile_pool(name="pst", bufs=2, space="PSUM") as ps_t,  # transposes
            tc.tile_pool(name="ps1", bufs=2, space="PSUM") as ps_1,  # m1 out
            tc.tile_pool(name="ps2", bufs=2, space="PSUM") as ps_2,  # m2 out
            tc.tile_pool(name="psA", bufs=2, space="PSUM") as ps_a,  # agg
        ):
            # ---- constants ----
            ident_f = cb.tile([P, P], F32)
            make_identity(nc, ident_f[:])
            ident_r = cb.tile([P, P], F32R)
            nc.vector.tensor_copy(ident_r[:], ident_f[:])

            iota_i = cb.tile([P, P], I32)
            nc.gpsimd.iota(iota_i[:], pattern=[[1, P]], base=0,
                           channel_multiplier=0)
            iota_f = cb.tile([P, P], F32)
            nc.vector.tensor_copy(iota_f[:], iota_i[:])

            # weights as lhsT layout [ki, ko, m] in f32r
            w1_f = cb.tile([P, 2, D], F32)
            nc.sync.dma_start(w1_f[:], w1_d.rearrange("(ko ki) m -> ki ko m",
                                                      ki=P))
            w1_r = cb.tile([P, 2, D], F32R)
            nc.vector.tensor_copy(w1_r[:], w1_f[:])
            w2_f = cb.tile([P, 2, D], F32)
            nc.sync.dma_start(w2_f[:], w2_d.rearrange("(ko ki) m -> ki ko m",
                                                      ki=P))
            w2_r = cb.tile([P, 2, D], F32R)
            nc.vector.tensor_copy(w2_r[:], w2_f[:])

            b1_sb = cb.tile([P, 2], F32)
            nc.sync.dma_start(b1_sb[:], b1_d.rearrange("(m p) -> p m", p=P))
            b2_sb = cb.tile([P, 2], F32)
            nc.sync.dma_start(b2_sb[:], b2_d.rearrange("(m p) -> p m", p=P))

            # all src / dstloc indices in one DMA each
            src_sb = cb.tile([P, 2 * n_tiles], I32)
            nc.sync.dma_start(src_sb[:], src_d)
            dst_sb = cb.tile([P, 2 * n_tiles], F32)
            nc.sync.dma_start(dst_sb[:], dst_d)

            t = 0  # global macrotile index
            for w in range(n_win):
                tw = tiles_per_window[w]
                # nodes [w*WIN, (w+1)*WIN) of this core's slice
                agg_ps = ps_a.tile([P, D], F32, name="agg_ps") if tw > 0 else None
                n_mm = 2 * tw  # scatter matmuls in this window's group
                mm_i = 0
                for _ in range(tw):
                    # ---- x = rel ; x += h[src] (gather-add) ----
                    x_sb = x_pool.tile([P, 2, D], F32)
                    nc.sync.dma_start(
                        x_sb[:],
                        rel_d[t * ET:(t + 1) * ET].rearrange(
                            "(j p) d -> p j d", p=P),
                    )
                    for j in range(2):
                        nc.gpsimd.indirect_dma_start(
                            out=x_sb[:, j],
                            out_offset=None,
                            in_=h_d,
                            in_offset=bass.IndirectOffsetOnAxis(
                                ap=src_sb[:, 2 * t + j:2 * t + j + 1], axis=0),
                            compute_op=mybir.AluOpType.add,
                        )

                    # ---- cast x to f32r, transpose x -> xT [ki, k, e] ----
                    xr_sb = x_pool.tile([P, 2, D], F32R, name="xr_sb",
                                        tag="xr")
                    nc.vector.tensor_copy(xr_sb[:], x_sb[:])
                    xT_ps = ps_t.tile([P, 2, ET], F32R, tag="trans")
                    for j in range(2):
                        for k in range(2):
                            nc.tensor.transpose(
                                xT_ps[:, k, j * P:(j + 1) * P],
                                xr_sb[:, j, k * P:(k + 1) * P],
                                ident_r[:],
                            )
                    xT_sb = xT_pool.tile([P, 2, ET], F32R)
                    nc.scalar.copy(xT_sb[:], xT_ps[:])

                    # ---- m1: hidT[m] = sum_k W1[k,m]^T @ xT[k] ----
                    hidT_ps = ps_1.tile([P, 2, ET], F32)
                    for m in range(2):
                        for k in range(2):
                            nc.tensor.matmul(
                                hidT_ps[:, m],
                                lhsT=w1_r[:, k, m * P:(m + 1) * P],
                                rhs=xT_sb[:, k],
                                start=(k == 0),
                                stop=(k == 1),
                            )
                    # relu(+b1) with f32r cast, per-partition bias
                    hidT_sb = hidT_pool.tile([P, 2, ET], F32R)
                    for m in range(2):
                        nc.scalar.activation(
                            hidT_sb[:, m], hidT_ps[:, m], AF.Relu,
                            bias=b1_sb[:, m:m + 1] if has_b1 else 0.0,
                        )

                    # ---- m2: msgT[m] = sum_k W2[k,m]^T @ hidT[k] ----
                    msgT_ps = ps_2.tile([P, 2, ET], F32)
                    for m in range(2):
                        for k in range(2):
                            nc.tensor.matmul(
                                msgT_ps[:, m],
                                lhsT=w2_r[:, k, m * P:(m + 1) * P],
                                rhs=hidT_sb[:, k],
                                start=(k == 0),
                                stop=(k == 1),
                            )
                    msgT_sb = msgT_pool.tile([P, 2, ET], F32R)
                    if has_b2:
                        for m in range(2):
                            nc.vector.tensor_scalar(
                                out=msgT_sb[:, m], in0=msgT_ps[:, m],
                                scalar1=b2_sb[:, m:m + 1], scalar2=None,
                                op0=mybir.AluOpType.add,
                            )
                    else:
                        nc.vector.tensor_copy(msgT_sb[:], msgT_ps[:])

                    # ---- transpose msgT -> msg [p=e(j), d] ----
                    msg_ps = ps_t.tile([P, 2, ET], F32R, tag="trans")
                    for j in range(2):
                        for k in range(2):
                            nc.tensor.transpose(
                                msg_ps[:, j, k * P:(k + 1) * P],
                                msgT_sb[:, k, j * P:(j + 1) * P],
                                ident_r[:],
                            )
                    msg_sb = msg_pool.tile([P, 2, ET], F32R)
                    nc.vector.tensor_copy(msg_sb[:], msg_ps[:])

                    # ---- selection matrices + scatter matmul ----
                    s_sb = s_pool.tile([P, 2, P], F32R)
                    for j in range(2):
                        nc.vector.tensor_scalar(
                            out=s_sb[:, j], in0=iota_f[:],
                            scalar1=dst_sb[:, 2 * t + j:2 * t + j + 1],
                            scalar2=None,
                            op0=mybir.AluOpType.is_equal,
                        )
                    for j in range(2):
                        nc.tensor.matmul(
                            agg_ps[:],
                            lhsT=s_sb[:, j],
                            rhs=msg_sb[:, j],
                            start=(mm_i == 0),
                            stop=(mm_i == n_mm - 1),
                            skip_group_check=True,
                        )
                        mm_i += 1
                    t += 1

                # ---- window close: out = h + agg ----
                h_sb = h_pool.tile([P, D], F32)
                nc.sync.dma_start(h_sb[:], hs_d[w * WIN:(w + 1) * WIN])
                out_sb = out_pool.tile([P, D], F32)
                if tw > 0:
                    nc.vector.tensor_add(out_sb[:], agg_ps[:], h_sb[:])
                else:
                    nc.vector.tensor_copy(out_sb[:], h_sb[:])
                nc.sync.dma_start(out_d[w * WIN:(w + 1) * WIN], out_sb[:])

    nc.compile()
    return nc


def _prepare_shards(h, src, dst, rel, n_nodes):
    """Shard + sort + pad edges by destination range. Returns per-core input
    arrays and the shared tiles_per_window schedule."""
    npc = n_nodes // N_CORES
    n_win = (npc + WIN - 1) // WIN
    npc_pad = n_win * WIN

    cores = []
    counts = np.zeros((N_CORES, n_win), dtype=np.int64)
    for c in range(N_CORES):
        lo, hi = c * npc, (c + 1) * npc
        mask = (dst >= lo) & (dst < hi)
        idx = np.nonzero(mask)[0]
        d_c = dst[idx] - lo
        order = np.argsort(d_c, kind="stable")
        idx = idx[order]
        d_c = d_c[order]
        w_c = d_c // WIN
        counts[c] = np.bincount(w_c, minlength=n_win)
        cores.append((idx, d_c, w_c))

    tiles_per_window = [
        max(1, int(-(-counts[:, w].max() // ET))) for w in range(n_win)
    ]
    n_tiles = int(sum(tiles_per_window))
    starts = np.concatenate([[0], np.cumsum(tiles_per_window)])

    in_maps = []
    for c in range(N_CORES):
        idx, d_c, w_c = cores[c]
        src_pad = np.zeros(n_tiles * ET, dtype=np.int32)
        dloc_pad = np.full(n_tiles * ET, -1.0, dtype=np.float32)
        rel_pad = np.zeros((n_tiles * ET, D), dtype=np.float32)
        bounds = np.searchsorted(w_c, np.arange(n_win + 1))
        for w in range(n_win):
            a, b = bounds[w], bounds[w + 1]
            k = b - a
            off = int(starts[w]) * ET
            src_pad[off:off + k] = src[idx[a:b]]
            dloc_pad[off:off + k] = (d_c[a:b] - w * WIN).astype(np.float32)
            rel_pad[off:off + k] = rel[idx[a:b]]
        h_slice = np.zeros((npc_pad, D), dtype=np.float32)
        h_slice[:npc] = h[c * npc:(c + 1) * npc]
        in_maps.append({
            "h": h,
            "h_slice": h_slice,
            "rel": rel_pad,
            "srcT": np.ascontiguousarray(src_pad.reshape(2 * n_tiles, P).T),
            "dstT": np.ascontiguousarray(dloc_pad.reshape(2 * n_tiles, P).T),
        })
    return in_maps, tiles_per_window, npc, n_win


def kernel(h, edge_index, edge_rel_emb, W1, b1, W2, b2, num_nodes):
    h = np.ascontiguousarray(h, dtype=np.float32)
    rel = np.ascontiguousarray(edge_rel_emb, dtype=np.float32)
    W1 = np.ascontiguousarray(W1, dtype=np.float32)
    W2 = np.ascontiguousarray(W2, dtype=np.float32)
    b1 = np.ascontiguousarray(b1, dtype=np.float32)
    b2 = np.ascontiguousarray(b2, dtype=np.float32)
    n_nodes = int(num_nodes)
    src = np.asarray(edge_index[0]).astype(np.int64)
    dst = np.asarray(edge_index[1]).astype(np.int64)
    assert n_nodes % N_CORES == 0
    assert h.shape == (n_nodes, D)

    in_maps, tiles_per_window, npc, n_win = _prepare_shards(
        h, src, dst, rel, n_nodes)

    has_b1 = bool(np.any(b1))
    has_b2 = bool(np.any(b2))
    key = (n_nodes, tuple(tiles_per_window), has_b1, has_b2)
    if key not in _CACHE:
        _CACHE[key] = _build_program(n_nodes, tiles_per_window, has_b1, has_b2)
    nc = _CACHE[key]

    for m in in_maps:
        m["w1"] = W1
        m["w2"] = W2
        m["b1"] = b1
        m["b2"] = b2

    trace_kwargs = {}
    if TRACE:
        trace_kwargs = dict(trace=True, tmpdir=TRACE_DIR,
                            trace_cores=list(range(N_CORES)))
    res = run_bass_kernel_spmd(nc, in_maps, core_ids=list(range(N_CORES)),
                               **trace_kwargs)
    out = np.concatenate(
        [res.results[c]["out"][:npc] for c in range(N_CORES)], axis=0)

    # stash for test harnesses
    kernel.last_results = res
    return out.astype(np.float32)
